# revision 23
# baseline (speedup 1.0000x reference)
"""BiLIF (bidirectional leaky-integrate-and-fire) node on 8 Trainium2 NeuronCores.

Problem: inputs [T=16, B=64, N=65536] f32.
  s1 = LIF-scan(x,          tau=4/3, v_th=0.75)   (hard reset to 0)
  s2 = LIF-scan(flip(x, 0), tau=4/3, v_th=1.25)
  out = (s1 + s2) / 2

Strategy (v2 -- int16 fixed-point scans in a hand-written 2x DVE uop):
  - Shard the batch dim across the 8 cores (pure data parallel). Per core:
    8*65536 positions = 128 partitions x 4096 columns, two 2048-col chunks.
  - Rescale the recurrence: u = h/0.75 obeys
        u_t = x_t + 0.25 * u_{t-1} * [u_{t-1} < th/0.75],
    spike_t = [u_t >= th/0.75]. Work in units of 1/4096: u_hat = 4096*u
    stays in [-29601, 29601] -> int16, with quantization step 2.44e-4
    (4x tighter than fp16 near the threshold ~5461). Measured flip count
    vs the fp32 reference: ~2.9e3 of 134M spikes -> rel err ~1.4e-2,
    safely under the 2e-2 gate (fp16 would be 2.1e-2 -- fails).
  - The scan step is a custom DVE op LIF2X with a hand-written 2x_1P
    perf-mode uop: two parallel 4-ALU chains (IS_LT, MUL, MUL, ADD)
    process two packed int16 elements per cycle (~860 ns per [128,2048]
    tile). Custom ops are otherwise capped at 1 elem/cycle (~2.1 us);
    this halves the dominant cost of the kernel.
  - x is converted fp32 -> int16 (exact *4096, RNE) once per tile on
    ScalarE (Copy w/ scale); ScalarE also does the PSUM->uint8 pack copy.
  - Spikes: builtin tensor_scalar is_ge on VectorE -- int16 in / bf16 out
    engages the stock 4x perf mode (~550 ns per tile; ScalarE Sign would
    be ~1.8 us and made ScalarE the bottleneck). The integer-valued state
    uses threshold 4095.5 for the u>=4096 test so Sign/is_ge ties cannot
    occur. TensorE packs 4 timesteps x both directions into one uint8
    digit-plane via scaled-identity matmuls accumulated in PSUM:
    out_byte = sum_j 4^j d_j, base-4 digits d_j = (s1+s2) in {0,1,2}.
    Output DMA shrinks 4x: 2 MB/core vs 8.
  - DMA per core: 32 MB in + 2 MB out =~ 89 us at the ~358 GB/s HBM-per-
    core limit -- the binding constraint. Engines: DVE ~85 us, ACT ~70,
    PE ~55. Measured 90-91 us/core vs 134 us for the fp32 1x baseline
    (1.47x) and 155.5 us for the original graded baseline (1.70x).
    Measured rel err 1.42e-2 (2842 flipped spikes of 134M) vs the 2e-2
    gate; fp16 state/input would be 2.1e-2 and fails.
"""

from dataclasses import dataclass

import numpy as np
import ml_dtypes  # noqa: F401

import concourse.bacc as bacc
import concourse.mybir as mybir
import concourse.tile as tile
import concourse.dve_ops as dve_ops
from concourse.dve_ops import DveOp
from concourse.dve_spec import (C0, C1, Spec, Src0, Src1, Zero, lower, select)
from concourse.dve_uop import (AluInp, AluOp, DelayInp, DveOpSpec, InpSel,
                               OutPath, OutSel, Trigger, UopConfig,
                               UopDpConfig)
from concourse import bass_isa, bass_utils
from concourse.masks import make_identity

T, B, N = 16, 64, 65536
NCORES = 8
BS = B // NCORES        # batch rows per core
P = 128
FREE = BS * N // P      # 4096 columns per partition
CHUNK = 2048
NCHUNK = FREE // CHUNK
NGROUP = 4              # timesteps packed per output byte
SC = 4096.0             # fixed-point scale (u_hat = 4096 * h / 0.75)
CC = 0.25               # recurrence coefficient in u-space
TH1H = float(np.float32(np.float32(0.75 / 0.75) * np.float32(SC)))   # 4096
TH2H = float(np.float32(np.float32(1.25 / 0.75) * np.float32(SC)))   # 6826.67
F32 = mybir.dt.float32
I16 = mybir.dt.int16
U8 = mybir.dt.uint8
BF16 = mybir.dt.bfloat16
AF = mybir.ActivationFunctionType


# --------------------------------------------------------------------------
# LIF2X: u' = x + s0 * select(u < s1, u, 0) with a hand-written 2x_1P uop.
# --------------------------------------------------------------------------

def _build_2x_uop() -> UopConfig:
    """Two packed 16-bit elements per cycle: chain-lo on stages 0-3,
    chain-hi on stages 4-7, operands staged through delay lanes."""
    u = UopConfig()
    u.enable_input(InpSel.SRC_1, 0)      # u_lo -> ALU lane
    u.enable_input(InpSel.SRC_0, 1)      # x_lo -> d0
    u.enable_input(InpSel.SRC_0_HI, 2)   # x_hi -> d1
    u.enable_input(InpSel.SRC_1_HI, 3)   # u_hi -> d2
    u.enable_input(InpSel.CONST_1, 4)    # s1 (threshold) -> d3
    u.enable_input(InpSel.CONST_0, 5)    # s0 (0.25) -> d4
    dp = u.datapath_config
    dp[0] = (UopDpConfig()
             .enable_alu(AluOp.IS_LT, AluInp.PREV_ALU_OUT, AluInp.PREV_DELAY_3)
             .enable_delay_from_src(DelayInp.PREV_ALU_OUT, 5)
             .pass_through_delay(0, 1, 2, 3, 4))
    dp[1] = (UopDpConfig()
             .enable_alu(AluOp.MULTIPLY, AluInp.PREV_ALU_OUT,
                         AluInp.PREV_DELAY_5)
             .pass_through_delay(0, 1, 2, 3, 4))
    dp[2] = (UopDpConfig()
             .enable_alu(AluOp.MULTIPLY, AluInp.PREV_ALU_OUT,
                         AluInp.PREV_DELAY_4)
             .pass_through_delay(0, 1, 2, 3, 4))
    dp[3] = (UopDpConfig()
             .enable_alu(AluOp.ADD, AluInp.PREV_ALU_OUT, AluInp.PREV_DELAY_0)
             .pass_through_delay(1, 2, 3, 4))
    dp[4] = (UopDpConfig()
             .enable_alu(AluOp.IS_LT, AluInp.PREV_DELAY_2, AluInp.PREV_DELAY_3)
             .enable_delay_from_src(DelayInp.PREV_ALU_OUT, 0)
             .pass_through_delay(1, 2, 4))
    dp[5] = (UopDpConfig()
             .enable_alu(AluOp.MULTIPLY, AluInp.PREV_ALU_OUT,
                         AluInp.PREV_DELAY_2)
             .pass_through_delay(0, 1, 4))
    dp[6] = (UopDpConfig()
             .enable_alu(AluOp.MULTIPLY, AluInp.PREV_ALU_OUT,
                         AluInp.PREV_DELAY_4)
             .pass_through_delay(0, 1))
    dp[7] = (UopDpConfig()
             .enable_alu(AluOp.ADD, AluInp.PREV_ALU_OUT, AluInp.PREV_DELAY_1)
             .pass_through_delay(0))
    u.enable_output(OutSel.DELAY_0, OutPath.WR0_LO)
    u.enable_output(OutSel.ALU_OUT, OutPath.WR0_HI)
    u.require_inp0 = 1
    u.require_inp1 = 1
    u.trigger = (Trigger.SRC_TENSOR_DONE, Trigger.NONE, Trigger.NONE)
    u.next_uop = (0, 0, 0)
    u.repeat_count = 0
    return u


def _spike_uops():
    """SPIKE2X: s = (u >= C0) -- hand uops for all four perf modes.
    Single-source: in 2-port modes the engine feeds the same tensor's next
    elements through the SRC_1 lanes; results are re-packed in element order
    across WR0_LO/HI (+ WR1_LO/HI at 4x)."""
    def base(u):
        u.require_inp0 = 1
        u.require_inp1 = 0
        u.trigger = (Trigger.SRC_TENSOR_DONE, Trigger.NONE, Trigger.NONE)
        u.next_uop = (0, 0, 0)
        u.repeat_count = 0
        return u

    # 1x: elem on ALU lane, C0 on d0
    u1 = base(UopConfig())
    u1.enable_input(InpSel.SRC_0, 0)
    u1.enable_input(InpSel.CONST_0, 1)
    u1.datapath_config[0] = (UopDpConfig()
                             .enable_alu(AluOp.IS_GE, AluInp.PREV_ALU_OUT,
                                         AluInp.PREV_DELAY_0))
    for i in range(1, 8):
        u1.datapath_config[i] = UopDpConfig()
        u1.datapath_config[i].alu_out_enable = 1
    u1.enable_output(OutSel.ALU_OUT, OutPath.WR0_LO)

    # 2x_1P: [e1|e0] packed on port 0
    u2 = base(UopConfig())
    u2.enable_input(InpSel.SRC_0, 0)        # e0 -> ALU lane
    u2.enable_input(InpSel.CONST_0, 1)      # th -> d0
    u2.enable_input(InpSel.SRC_0_HI, 2)     # e1 -> d1
    dp = u2.datapath_config
    dp[0] = (UopDpConfig()
             .enable_alu(AluOp.IS_GE, AluInp.PREV_ALU_OUT, AluInp.PREV_DELAY_0)
             .pass_through_delay(0, 1))
    dp[1] = (UopDpConfig()
             .enable_alu(AluOp.IS_GE, AluInp.PREV_DELAY_1, AluInp.PREV_DELAY_0)
             .enable_delay_from_src(DelayInp.PREV_ALU_OUT, 2))
    for i in range(2, 8):
        dp[i] = UopDpConfig().pass_through_delay(2)
        dp[i].alu_out_enable = 1
    u2.enable_output(OutSel.DELAY_2, OutPath.WR0_LO)
    u2.enable_output(OutSel.ALU_OUT, OutPath.WR0_HI)

    # 2x_2P: e0 on port0 (SRC_0), e1 on port1 (SRC_1)
    u2p = base(UopConfig())
    u2p.enable_input(InpSel.SRC_0, 0)
    u2p.enable_input(InpSel.CONST_0, 1)
    u2p.enable_input(InpSel.SRC_1, 2)
    dp = u2p.datapath_config
    dp[0] = (UopDpConfig()
             .enable_alu(AluOp.IS_GE, AluInp.PREV_ALU_OUT, AluInp.PREV_DELAY_0)
             .pass_through_delay(0, 1))
    dp[1] = (UopDpConfig()
             .enable_alu(AluOp.IS_GE, AluInp.PREV_DELAY_1, AluInp.PREV_DELAY_0)
             .enable_delay_from_src(DelayInp.PREV_ALU_OUT, 2))
    for i in range(2, 8):
        dp[i] = UopDpConfig().pass_through_delay(2)
        dp[i].alu_out_enable = 1
    u2p.enable_output(OutSel.DELAY_2, OutPath.WR0_LO)
    u2p.enable_output(OutSel.ALU_OUT, OutPath.WR0_HI)

    # 4x: e0..e3 = SRC_0, SRC_0_HI, SRC_1, SRC_1_HI
    u4 = base(UopConfig())
    u4.enable_input(InpSel.SRC_0, 0)        # e0 -> ALU lane
    u4.enable_input(InpSel.CONST_0, 1)      # th -> d0
    u4.enable_input(InpSel.SRC_0_HI, 2)     # e1 -> d1
    u4.enable_input(InpSel.SRC_1, 3)        # e2 -> d2
    u4.enable_input(InpSel.SRC_1_HI, 4)     # e3 -> d3
    dp = u4.datapath_config
    dp[0] = (UopDpConfig()
             .enable_alu(AluOp.IS_GE, AluInp.PREV_ALU_OUT, AluInp.PREV_DELAY_0)
             .pass_through_delay(0, 1, 2, 3))
    dp[1] = (UopDpConfig()
             .enable_alu(AluOp.IS_GE, AluInp.PREV_DELAY_1, AluInp.PREV_DELAY_0)
             .enable_delay_from_src(DelayInp.PREV_ALU_OUT, 4)
             .pass_through_delay(0, 2, 3))
    dp[2] = (UopDpConfig()
             .enable_alu(AluOp.IS_GE, AluInp.PREV_DELAY_2, AluInp.PREV_DELAY_0)
             .enable_delay_from_src(DelayInp.PREV_ALU_OUT, 5)
             .pass_through_delay(0, 3, 4))
    dp[3] = (UopDpConfig()
             .enable_alu(AluOp.IS_GE, AluInp.PREV_DELAY_3, AluInp.PREV_DELAY_0)
             .enable_delay_from_src(DelayInp.PREV_ALU_OUT, 0)
             .pass_through_delay(4, 5))
    for i in range(4, 8):
        dp[i] = UopDpConfig().pass_through_delay(0, 4, 5)
        dp[i].alu_out_enable = 1
    u4.enable_output(OutSel.DELAY_4, OutPath.WR0_LO)   # e0 result
    u4.enable_output(OutSel.DELAY_5, OutPath.WR0_HI)   # e1 result
    u4.enable_output(OutSel.DELAY_0, OutPath.WR1_LO)   # e2 result
    u4.enable_output(OutSel.ALU_OUT, OutPath.WR1_HI)   # e3 result
    return u1, u2, u2p, u4


@dataclass(frozen=True)
class _DveOpHand(DveOp):
    """DveOp whose compiled DveOpSpec is prebuilt (with perf-mode uops)."""
    handspec: DveOpSpec | None = None

    def compile(self, ver):
        assert ver == "v3", "hand uop built for v3/TRN2 only"
        return self.handspec


_LIF_SPEC = Spec(
    body=Src0 + select(Src1 < C1, Src1, Zero) * C0,
    reference=lambda in0, in1, s0, s1, imm2: (
        in0.astype(np.float32)
        + np.where(in1.astype(np.float32) < s1,
                   in1.astype(np.float32), np.float32(0)) * np.float32(s0)
    ),
)


_SPIKE_SPEC = Spec(
    body=Src0 >= C0,
    reference=lambda in0, in1, s0, s1, imm2: (
        in0.astype(np.float32) >= s0).astype(np.float32),
)


def _register_hand(name) -> DveOp:
    if name in dve_ops._SUB_OPCODE_FOR_NAME:
        for op in dve_ops.OPS:
            if op.name == name:
                return op
    row = dve_ops._CUSTOM_DVE_ROW_BASE + len(dve_ops.OPS)
    assert row < 0x20, "custom DVE opcode rows exhausted"
    if name == "LIF2X":
        spec, rd1 = _LIF_SPEC, True
        hs = DveOpSpec(name=name, opcode=row, uops=lower(spec, ver="v3"),
                       uops_2x=[_build_2x_uop()], perf_max=1, rd1_en=True)
    else:
        # perf_max=1: the 2-port single-src modes (2x_2P/4x) hang on HW --
        # the CUSTOM_DVE_ANT instruction carries no src1 AP for the engine
        # to drive port 1 with. 2x_1P is validated.
        spec, rd1 = _SPIKE_SPEC, False
        u1, u2, u2p, u4 = _spike_uops()
        hs = DveOpSpec(name=name, opcode=row, uops=[u1], uops_2x=[u2],
                       perf_max=1, rd1_en=False)
    op = _DveOpHand(name, spec, subdim=False, uops_sha={}, handspec=hs)
    dve_ops.OPS.append(op)
    dve_ops._SUB_OPCODE_FOR_NAME[name] = row
    dve_ops.CUSTOM_DVE_SPECS[name] = spec
    return op


def _emit_hand(vec, op, *, out, in0, in1=None, s0=0.0, s1=0.0):
    """nc.vector._custom_dve with the op's perf_max on the instruction."""
    bass = vec.bass
    if op.name not in bass.m.ant_custom_dve_ops:
        bass.m.ant_custom_dve_ops = sorted(
            {*bass.m.ant_custom_dve_ops, op.name})
    from concourse.dve_ops import get_dve_sub_opcode
    shape = bass_isa.CustomDveShape.TTSS
    isa_opcode = bass.isa.Opcode[
        f"NEURON_ISA_TPB_OPCODE_CUSTOM_DVE_ANT_{shape.slot()}"].value

    def imm(v):
        return mybir.ImmediateValue(dtype=mybir.dt.float32, value=float(v))

    ins = [vec.lower_ap(in0, for_isa=True, opt=True)]
    if in1 is not None:
        ins.append(vec.lower_ap(in1, for_isa=True, opt=True))
    ins += [imm(s0), imm(s1)]
    outs = [vec.lower_ap(out, for_isa=True, opt=True)]
    return vec.add_instruction(
        bass_isa.InstCustomDveAnt(
            name=bass.get_next_instruction_name(),
            op_name=op.name,
            rd1_en=in1 is not None,
            subdim=0,
            imm2=0.0,
            shape=shape,
            row=get_dve_sub_opcode(op.name),
            isa_opcode=isa_opcode,
            ins=ins,
            outs=outs,
            perf_max=op.handspec.perf_max,
        )
    )


# --------------------------------------------------------------------------
# Per-core program
# --------------------------------------------------------------------------

_NC_CACHE = {}


def _build_nc(repeat: int = 1, mode: str = "full", conv: str = "act",
              sign: str = "bdve"):
    """mode: full | nosign (skip signs+pack) | noscan (loads+converts only).
    conv: split | dve | act  -- where the fp32->int16 convert runs.
    sign: act (ScalarE Sign, +-1) | dve (SPIKE2X is_ge, {0,1})
          | mix (dir1 on DVE, dir2 on ACT)."""
    key = (repeat, mode, conv, sign)
    if key in _NC_CACHE:
        return _NC_CACHE[key]
    op = _register_hand("LIF2X")
    spk = _register_hand("SPIKE2X")
    nc = bacc.Bacc("TRN2", target_bir_lowering=False, debug=False,
                   num_devices=NCORES)
    x_d = nc.dram_tensor("x", [T * NCHUNK * P, CHUNK], F32,
                         kind="ExternalInput").ap()
    o_d = nc.dram_tensor("o", [NGROUP * P, FREE], U8,
                         kind="ExternalOutput").ap()

    ub = 2 if sign == "pair" else 3
    gb = 4 if sign == "pair" else 5
    xb = 3 if sign == "pair" else 4
    hb = 17 if sign == "pair" else 18
    with tile.TileContext(nc) as tc:
        with tc.tile_pool(name="xf", bufs=xb) as xfp, \
             tc.tile_pool(name="xh", bufs=hb) as xhp, \
             tc.tile_pool(name="u1", bufs=ub) as u1p, \
             tc.tile_pool(name="u2", bufs=ub) as u2p, \
             tc.tile_pool(name="g1", bufs=gb) as g1p, \
             tc.tile_pool(name="g2", bufs=gb) as g2p, \
             tc.tile_pool(name="ou", bufs=3) as oup, \
             tc.tile_pool(name="ps", bufs=2, space="PSUM") as psp, \
             tc.tile_pool(name="cn", bufs=1) as cnp:
            # constants: scaled identities 4^j (and half-weights for the
            # +-1-sign path), sign biases
            ident = cnp.tile([P, P], BF16, tag="id", name="id")
            make_identity(nc, ident[:])
            idw, idh = [ident], []
            for j in (1, 2, 3):
                t_ = cnp.tile([P, P], BF16, tag=f"id{j}", name=f"id{j}")
                nc.vector.tensor_scalar(out=t_[:], in0=ident[:],
                                        scalar1=float(4 ** j), scalar2=None,
                                        op0=mybir.AluOpType.mult)
                idw.append(t_)
            for j in range(4):
                t_ = cnp.tile([P, P], BF16, tag=f"ih{j}", name=f"ih{j}")
                nc.vector.tensor_scalar(out=t_[:], in0=ident[:],
                                        scalar1=float(4 ** j) / 2,
                                        scalar2=None,
                                        op0=mybir.AluOpType.mult)
                idh.append(t_)
            # -0.5 shift: u_hat is integer-valued, so a half-step offset makes
            # Sign() tie-free ([u >= 4096] == [u >= 4095.5]) without changing
            # the decision; exact ties at integer thresholds would emit
            # sign=0 and corrupt the packed base-4 digits.
            b1 = cnp.tile([P, 1], F32, tag="b1", name="b1")
            nc.vector.memset(b1[:], -(TH1H - 0.5))
            b2 = cnp.tile([P, 1], F32, tag="b2", name="b2")
            nc.vector.memset(b2[:], -TH2H)

            for rep in range(repeat):
                for k in range(NCHUNK):
                    c0 = k * CHUNK
                    # load + convert, in first-use order
                    xh = {}
                    for i, t in enumerate(
                            v for s in range(T // 2) for v in (s, T - 1 - s)):
                        xf = xfp.tile([P, CHUNK], F32, tag="xf",
                                      name=f"xf{rep}_{k}_{t}")
                        r0 = (t * NCHUNK + k) * P
                        nc.sync.dma_start(
                            out=xf[:], in_=x_d[r0:r0 + P, :])
                        xh[t] = xhp.tile([P, CHUNK], I16, tag="xh",
                                         name=f"xh{rep}_{k}_{t}")
                        on_dve = (conv == "dve" or
                                  (conv == "split" and i % 2 == 0))
                        if on_dve:
                            nc.vector.tensor_scalar(
                                out=xh[t][:], in0=xf[:], scalar1=SC,
                                scalar2=None, op0=mybir.AluOpType.mult)
                        else:
                            nc.scalar.activation(
                                out=xh[t][:], in_=xf[:], func=AF.Copy,
                                bias=0.0, scale=SC)
                    if mode == "noscan":
                        ob = oup.tile([P, CHUNK], U8, tag="ou", name="ou")
                        nc.vector.tensor_scalar(
                            out=ob[:, :64], in0=xh[0][:, :64], scalar1=1.0,
                            scalar2=None, op0=mybir.AluOpType.mult)
                        nc.sync.dma_start(out=o_d[:P, c0:c0 + 64],
                                          in_=ob[:, :64])
                        continue
                    if sign == "pair":
                        # double-width state tiles; one FD-4096 is_ge covers
                        # two timesteps' spikes per direction.
                        sgA, sgB = {}, {}
                        packed = set()

                        def spike(dst, src, th):
                            nc.vector.tensor_scalar(
                                out=dst, in0=src, scalar1=th, scalar2=None,
                                op0=mybir.AluOpType.is_ge)

                        def try_pack():
                            for g in range(NGROUP):
                                if g in packed:
                                    continue
                                ts_ = range(g * NGROUP, (g + 1) * NGROUP)
                                if not all(q in sgA and q in sgB
                                           for q in ts_):
                                    continue
                                packed.add(g)
                                ps = psp.tile([P, CHUNK], F32, tag="ps",
                                              name="ps")
                                for blk in range(CHUNK // 512):
                                    for j in range(NGROUP):
                                        tj = g * NGROUP + j
                                        for dmap, st_, sp_ in (
                                                (sgA, j == 0, False),
                                                (sgB, False,
                                                 j == NGROUP - 1)):
                                            tl, off = dmap[tj]
                                            b0 = off + blk * 512
                                            nc.tensor.matmul(
                                                ps[:, blk * 512:
                                                   (blk + 1) * 512],
                                                idw[j][:],
                                                tl[:, b0:b0 + 512],
                                                start=st_, stop=sp_)
                                ob = oup.tile([P, CHUNK], U8, tag="ou",
                                              name="ou")
                                nc.scalar.activation(
                                    out=ob[:], in_=ps[:], func=AF.Copy,
                                    bias=0.0, scale=1.0)
                                nc.sync.dma_start(
                                    out=o_d[g * P:(g + 1) * P,
                                            c0:c0 + CHUNK],
                                    in_=ob[:])

                        u1ap, u2ap = xh[0][:], xh[T - 1][:]
                        sa = g1p.tile([P, 2 * CHUNK], BF16, tag="g1",
                                      name="g1s0")
                        spike(sa[:, :CHUNK], u1ap, TH1H - 0.5)
                        sgA[0] = (sa, 0)
                        sb = g2p.tile([P, 2 * CHUNK], BF16, tag="g2",
                                      name="g2s0")
                        spike(sb[:, :CHUNK], u2ap, TH2H)
                        sgB[0] = (sb, 0)
                        p1 = p2 = None
                        for t in range(1, T):
                            half = (t - 1) % 2
                            if t == T - 1 or half == 0:
                                p1 = u1p.tile([P, 2 * CHUNK], I16, tag="u1",
                                              name="u1")
                                p2 = u2p.tile([P, 2 * CHUNK], I16, tag="u2",
                                              name="u2")
                                d1, d2 = p1[:, :CHUNK], p2[:, :CHUNK]
                            else:
                                d1, d2 = p1[:, CHUNK:], p2[:, CHUNK:]
                            _emit_hand(nc.vector, op, out=d1, in0=xh[t][:],
                                       in1=u1ap, s0=CC, s1=TH1H)
                            _emit_hand(nc.vector, op, out=d2,
                                       in0=xh[T - 1 - t][:], in1=u2ap,
                                       s0=CC, s1=TH2H)
                            u1ap, u2ap = d1, d2
                            if t == T - 1:
                                sa = g1p.tile([P, 2 * CHUNK], BF16,
                                              tag="g1", name="g1f")
                                spike(sa[:, :CHUNK], d1, TH1H - 0.5)
                                sgA[t] = (sa, 0)
                                sb = g2p.tile([P, 2 * CHUNK], BF16,
                                              tag="g2", name="g2f")
                                spike(sb[:, :CHUNK], d2, TH2H)
                                sgB[t] = (sb, 0)
                            elif half == 1:
                                sa = g1p.tile([P, 2 * CHUNK], BF16,
                                              tag="g1", name="g1")
                                spike(sa[:, :], p1[:, :], TH1H - 0.5)
                                sgA[t - 1] = (sa, 0)
                                sgA[t] = (sa, CHUNK)
                                sb = g2p.tile([P, 2 * CHUNK], BF16,
                                              tag="g2", name="g2")
                                spike(sb[:, :], p2[:, :], TH2H)
                                sgB[t - 1] = (sb, 0)
                                sgB[t] = (sb, CHUNK)
                            try_pack()
                        continue
                    # scans + signs; u at t=0 is the converted x tile itself
                    sg1, sg2 = {}, {}
                    u1, u2 = xh[0], xh[T - 1]
                    for t in range(T):
                        if t > 0:
                            u1n = u1p.tile([P, CHUNK], I16, tag="u1",
                                           name="u1")
                            u2n = u2p.tile([P, CHUNK], I16, tag="u2",
                                           name="u2")
                            _emit_hand(nc.vector, op, out=u1n[:],
                                       in0=xh[t][:], in1=u1[:],
                                       s0=CC, s1=TH1H)
                            _emit_hand(nc.vector, op, out=u2n[:],
                                       in0=xh[T - 1 - t][:], in1=u2[:],
                                       s0=CC, s1=TH2H)
                            u1, u2 = u1n, u2n
                        if mode == "nosign":
                            if t == T - 1:
                                ob = oup.tile([P, CHUNK], U8, tag="ou",
                                              name="ou")
                                nc.vector.tensor_tensor(
                                    out=ob[:, :64], in0=u1[:, :64],
                                    in1=u2[:, :64],
                                    op=mybir.AluOpType.add)
                                nc.sync.dma_start(out=o_d[:P, c0:c0 + 64],
                                                  in_=ob[:, :64])
                            continue
                        d1_dve = sign in ("dve", "mix", "bdve", "bmix",
                                          "gmix", "gmix2", "tune")
                        # tune: dir2 spikes of group 1 go to ACT Sign (+-1
                        # digits handled by per-group weights + copy bias).
                        d2_dve = (sign in ("dve", "bdve", "gmix", "gmix2") or
                                  (sign == "tune" and t // NGROUP != 1))
                        builtin = sign in ("bdve", "bmix", "gmix", "gmix2",
                                           "tune")

                        def emit_spike(dst, src, th, eng=None):
                            if builtin:
                                (eng or nc.vector).tensor_scalar(
                                    out=dst, in0=src, scalar1=th,
                                    scalar2=None,
                                    op0=mybir.AluOpType.is_ge)
                            else:
                                _emit_hand(nc.vector, spk, out=dst,
                                           in0=src, s0=th)
                        sg1[t] = g1p.tile([P, CHUNK], BF16, tag="g1",
                                          name="g1")
                        if d1_dve:
                            emit_spike(sg1[t][:], u1[:], TH1H - 0.5)
                        else:
                            nc.scalar.activation(out=sg1[t][:], in_=u1[:],
                                                 func=AF.Sign, bias=b1[:],
                                                 scale=1.0)
                        sg2[t] = g2p.tile([P, CHUNK], BF16, tag="g2",
                                          name="g2")
                        if sign == "gmix" or (sign == "gmix2" and t % 2 == 0):
                            emit_spike(sg2[t][:], u2[:], TH2H, nc.gpsimd)
                        elif d2_dve:
                            emit_spike(sg2[t][:], u2[:], TH2H)
                        else:
                            nc.scalar.activation(out=sg2[t][:], in_=u2[:],
                                                 func=AF.Sign, bias=b2[:],
                                                 scale=1.0)
                        if t % NGROUP == NGROUP - 1:
                            g = t // NGROUP
                            g2_dve = (sign in ("dve", "bdve", "gmix",
                                               "gmix2") or
                                      (sign == "tune" and g != 1))
                            w1 = idw if d1_dve else idh
                            w2 = idw if g2_dve else idh
                            obias = (0.0 if d1_dve else 42.5) + \
                                (0.0 if g2_dve else 42.5)
                            ps = psp.tile([P, CHUNK], F32, tag="ps",
                                          name="ps")
                            for blk in range(CHUNK // 512):
                                sl = slice(blk * 512, (blk + 1) * 512)
                                for j in range(NGROUP):
                                    tj = g * NGROUP + j
                                    nc.tensor.matmul(
                                        ps[:, sl], w1[j][:], sg1[tj][:, sl],
                                        start=(j == 0), stop=False)
                                    nc.tensor.matmul(
                                        ps[:, sl], w2[j][:], sg2[tj][:, sl],
                                        start=False, stop=(j == NGROUP - 1))
                            ob = oup.tile([P, CHUNK], U8, tag="ou", name="ou")
                            nc.scalar.activation(out=ob[:], in_=ps[:],
                                                 func=AF.Copy, bias=obias,
                                                 scale=1.0)
                            nc.sync.dma_start(
                                out=o_d[g * P:(g + 1) * P, c0:c0 + CHUNK],
                                in_=ob[:])

    nc.compile()
    _NC_CACHE[key] = nc
    return nc


def make_shard(inputs: np.ndarray, c: int) -> np.ndarray:
    """Per-core DRAM image: [T*NCHUNK*P, CHUNK] fp32 with each (t, chunk)
    tile a fully contiguous 1 MB block (contiguous DMA bursts)."""
    return np.ascontiguousarray(
        inputs[:, c * BS:(c + 1) * BS, :]
        .reshape(T, P, NCHUNK, CHUNK).transpose(0, 2, 1, 3)
        .reshape(T * NCHUNK * P, CHUNK))


def _run(inputs: np.ndarray, repeat: int = 1, **kwargs):
    nc = _build_nc(repeat)
    in_maps = []
    for c in range(NCORES):
        in_maps.append({"x": make_shard(inputs, c)})
    return bass_utils.run_bass_kernel_spmd(
        nc, in_maps, core_ids=list(range(NCORES)), **kwargs)


# digit-plane decode LUTs: byte -> 0.5 * base-4 digit j
_LUT = [((np.arange(256, dtype=np.uint8) >> (2 * j)) & 3).astype(np.float32)
        * np.float32(0.5) for j in range(NGROUP)]


def kernel(inputs: np.ndarray, **kwargs) -> np.ndarray:
    inputs = np.asarray(inputs)
    assert inputs.shape == (T, B, N) and inputs.dtype == np.float32
    res = None
    err = None
    for _attempt in range(3):  # retry transient device faults
        try:
            res = _run(inputs, **kwargs)
            break
        except Exception as e:  # noqa: BLE001
            err = e
    if res is None:
        raise err
    out = np.empty((T, B, N), np.float32)
    for c in range(NCORES):
        o = res.results[c]["o"]  # [NGROUP*P, FREE] uint8
        for g in range(NGROUP):
            plane = o[g * P:(g + 1) * P, :]
            for j in range(NGROUP):
                t = g * NGROUP + j
                out[t, c * BS:(c + 1) * BS, :] = (
                    _LUT[j][plane].reshape(BS, N))
    return out


# revision 26
# speedup vs baseline: 1.2207x; 1.2207x over previous
"""BiLIF (bidirectional leaky-integrate-and-fire) node on 8 Trainium2 NeuronCores.

Problem: inputs [T=16, B=64, N=65536] f32.
  s1 = LIF-scan(x,          tau=4/3, v_th=0.75)   (hard reset to 0)
  s2 = LIF-scan(flip(x, 0), tau=4/3, v_th=1.25)
  out = (s1 + s2) / 2

Strategy (v2 -- int16 fixed-point scans in a hand-written 2x DVE uop):
  - Shard the batch dim across the 8 cores (pure data parallel). Per core:
    8*65536 positions = 128 partitions x 4096 columns, two 2048-col chunks.
  - Rescale the recurrence: u = h/0.75 obeys
        u_t = x_t + 0.25 * u_{t-1} * [u_{t-1} < th/0.75],
    spike_t = [u_t >= th/0.75]. Work in units of 1/4096: u_hat = 4096*u
    stays in [-29601, 29601] -> int16, with quantization step 2.44e-4
    (4x tighter than fp16 near the threshold ~5461). Measured flip count
    vs the fp32 reference: ~2.9e3 of 134M spikes -> rel err ~1.4e-2,
    safely under the 2e-2 gate (fp16 would be 2.1e-2 -- fails).
  - The scan step is a custom DVE op LIF2X with a hand-written 2x_1P
    perf-mode uop: two parallel 4-ALU chains (IS_LT, MUL, MUL, ADD)
    process two packed int16 elements per cycle (~860 ns per [128,2048]
    tile). Custom ops are otherwise capped at 1 elem/cycle (~2.1 us);
    this halves the dominant cost of the kernel.
  - x is converted fp32 -> int16 (exact *4096, RNE) once per tile on
    ScalarE (Copy w/ scale); ScalarE also does the PSUM->uint8 pack copy.
  - Spikes: builtin tensor_scalar is_ge on VectorE -- int16 in / bf16 out
    engages the stock 4x perf mode (~550 ns per tile; ScalarE Sign would
    be ~1.8 us and made ScalarE the bottleneck). The integer-valued state
    uses threshold 4095.5 for the u>=4096 test so Sign/is_ge ties cannot
    occur. TensorE packs 4 timesteps x both directions into one uint8
    digit-plane via scaled-identity matmuls accumulated in PSUM:
    out_byte = sum_j 4^j d_j, base-4 digits d_j = (s1+s2) in {0,1,2}.
    Output DMA shrinks 4x: 2 MB/core vs 8.
  - The per-core DRAM image is resharded on host so every (t, chunk)
    input tile is one contiguous 1 MB block (single-burst DMA loads).
  - DMA per core: 32 MB in + 2 MB out =~ 85 us at the ~358 GB/s HBM-per-
    core limit; VectorE ~85 us (scans at the 2x-mode cap for a 4-ALU
    body, spikes at the stock 4x cap); ScalarE ~70; TensorE ~55. The
    machine is at its three-way floor. Measured 85.7-87.7 us/core under
    quiet conditions (vs 134 us for the fp32 1x baseline and 155.5 us
    for the original graded baseline); late-session readings of
    103-119 us occurred on bit-identical code and are host interference.
    Measured rel err 1.42e-2 (2842 flipped spikes of 134M) vs the 2e-2
    gate; fp16 state/input would be 2.1e-2 and fails.
"""

from dataclasses import dataclass

import numpy as np
import ml_dtypes  # noqa: F401

import concourse.bacc as bacc
import concourse.mybir as mybir
import concourse.tile as tile
import concourse.dve_ops as dve_ops
from concourse.dve_ops import DveOp
from concourse.dve_spec import (C0, C1, Spec, Src0, Src1, Zero, lower, select)
from concourse.dve_uop import (AluInp, AluOp, DelayInp, DveOpSpec, InpSel,
                               OutPath, OutSel, Trigger, UopConfig,
                               UopDpConfig)
from concourse import bass_isa, bass_utils
from concourse.masks import make_identity

T, B, N = 16, 64, 65536
NCORES = 8
BS = B // NCORES        # batch rows per core
P = 128
FREE = BS * N // P      # 4096 columns per partition
CHUNK = 2048
NCHUNK = FREE // CHUNK
NGROUP = 4              # timesteps packed per output byte
SC = 4096.0             # fixed-point scale (u_hat = 4096 * h / 0.75)
CC = 0.25               # recurrence coefficient in u-space
TH1H = float(np.float32(np.float32(0.75 / 0.75) * np.float32(SC)))   # 4096
TH2H = float(np.float32(np.float32(1.25 / 0.75) * np.float32(SC)))   # 6826.67
F32 = mybir.dt.float32
I16 = mybir.dt.int16
U8 = mybir.dt.uint8
BF16 = mybir.dt.bfloat16
AF = mybir.ActivationFunctionType


# --------------------------------------------------------------------------
# LIF2X: u' = x + s0 * select(u < s1, u, 0) with a hand-written 2x_1P uop.
# --------------------------------------------------------------------------

def _build_2x_uop() -> UopConfig:
    """Two packed 16-bit elements per cycle: chain-lo on stages 0-3,
    chain-hi on stages 4-7, operands staged through delay lanes."""
    u = UopConfig()
    u.enable_input(InpSel.SRC_1, 0)      # u_lo -> ALU lane
    u.enable_input(InpSel.SRC_0, 1)      # x_lo -> d0
    u.enable_input(InpSel.SRC_0_HI, 2)   # x_hi -> d1
    u.enable_input(InpSel.SRC_1_HI, 3)   # u_hi -> d2
    u.enable_input(InpSel.CONST_1, 4)    # s1 (threshold) -> d3
    u.enable_input(InpSel.CONST_0, 5)    # s0 (0.25) -> d4
    dp = u.datapath_config
    dp[0] = (UopDpConfig()
             .enable_alu(AluOp.IS_LT, AluInp.PREV_ALU_OUT, AluInp.PREV_DELAY_3)
             .enable_delay_from_src(DelayInp.PREV_ALU_OUT, 5)
             .pass_through_delay(0, 1, 2, 3, 4))
    dp[1] = (UopDpConfig()
             .enable_alu(AluOp.MULTIPLY, AluInp.PREV_ALU_OUT,
                         AluInp.PREV_DELAY_5)
             .pass_through_delay(0, 1, 2, 3, 4))
    dp[2] = (UopDpConfig()
             .enable_alu(AluOp.MULTIPLY, AluInp.PREV_ALU_OUT,
                         AluInp.PREV_DELAY_4)
             .pass_through_delay(0, 1, 2, 3, 4))
    dp[3] = (UopDpConfig()
             .enable_alu(AluOp.ADD, AluInp.PREV_ALU_OUT, AluInp.PREV_DELAY_0)
             .pass_through_delay(1, 2, 3, 4))
    dp[4] = (UopDpConfig()
             .enable_alu(AluOp.IS_LT, AluInp.PREV_DELAY_2, AluInp.PREV_DELAY_3)
             .enable_delay_from_src(DelayInp.PREV_ALU_OUT, 0)
             .pass_through_delay(1, 2, 4))
    dp[5] = (UopDpConfig()
             .enable_alu(AluOp.MULTIPLY, AluInp.PREV_ALU_OUT,
                         AluInp.PREV_DELAY_2)
             .pass_through_delay(0, 1, 4))
    dp[6] = (UopDpConfig()
             .enable_alu(AluOp.MULTIPLY, AluInp.PREV_ALU_OUT,
                         AluInp.PREV_DELAY_4)
             .pass_through_delay(0, 1))
    dp[7] = (UopDpConfig()
             .enable_alu(AluOp.ADD, AluInp.PREV_ALU_OUT, AluInp.PREV_DELAY_1)
             .pass_through_delay(0))
    u.enable_output(OutSel.DELAY_0, OutPath.WR0_LO)
    u.enable_output(OutSel.ALU_OUT, OutPath.WR0_HI)
    u.require_inp0 = 1
    u.require_inp1 = 1
    u.trigger = (Trigger.SRC_TENSOR_DONE, Trigger.NONE, Trigger.NONE)
    u.next_uop = (0, 0, 0)
    u.repeat_count = 0
    return u


def _spike_uops():
    """SPIKE2X: s = (u >= C0) -- hand uops for all four perf modes.
    Single-source: in 2-port modes the engine feeds the same tensor's next
    elements through the SRC_1 lanes; results are re-packed in element order
    across WR0_LO/HI (+ WR1_LO/HI at 4x)."""
    def base(u):
        u.require_inp0 = 1
        u.require_inp1 = 0
        u.trigger = (Trigger.SRC_TENSOR_DONE, Trigger.NONE, Trigger.NONE)
        u.next_uop = (0, 0, 0)
        u.repeat_count = 0
        return u

    # 1x: elem on ALU lane, C0 on d0
    u1 = base(UopConfig())
    u1.enable_input(InpSel.SRC_0, 0)
    u1.enable_input(InpSel.CONST_0, 1)
    u1.datapath_config[0] = (UopDpConfig()
                             .enable_alu(AluOp.IS_GE, AluInp.PREV_ALU_OUT,
                                         AluInp.PREV_DELAY_0))
    for i in range(1, 8):
        u1.datapath_config[i] = UopDpConfig()
        u1.datapath_config[i].alu_out_enable = 1
    u1.enable_output(OutSel.ALU_OUT, OutPath.WR0_LO)

    # 2x_1P: [e1|e0] packed on port 0
    u2 = base(UopConfig())
    u2.enable_input(InpSel.SRC_0, 0)        # e0 -> ALU lane
    u2.enable_input(InpSel.CONST_0, 1)      # th -> d0
    u2.enable_input(InpSel.SRC_0_HI, 2)     # e1 -> d1
    dp = u2.datapath_config
    dp[0] = (UopDpConfig()
             .enable_alu(AluOp.IS_GE, AluInp.PREV_ALU_OUT, AluInp.PREV_DELAY_0)
             .pass_through_delay(0, 1))
    dp[1] = (UopDpConfig()
             .enable_alu(AluOp.IS_GE, AluInp.PREV_DELAY_1, AluInp.PREV_DELAY_0)
             .enable_delay_from_src(DelayInp.PREV_ALU_OUT, 2))
    for i in range(2, 8):
        dp[i] = UopDpConfig().pass_through_delay(2)
        dp[i].alu_out_enable = 1
    u2.enable_output(OutSel.DELAY_2, OutPath.WR0_LO)
    u2.enable_output(OutSel.ALU_OUT, OutPath.WR0_HI)

    # 2x_2P: e0 on port0 (SRC_0), e1 on port1 (SRC_1)
    u2p = base(UopConfig())
    u2p.enable_input(InpSel.SRC_0, 0)
    u2p.enable_input(InpSel.CONST_0, 1)
    u2p.enable_input(InpSel.SRC_1, 2)
    dp = u2p.datapath_config
    dp[0] = (UopDpConfig()
             .enable_alu(AluOp.IS_GE, AluInp.PREV_ALU_OUT, AluInp.PREV_DELAY_0)
             .pass_through_delay(0, 1))
    dp[1] = (UopDpConfig()
             .enable_alu(AluOp.IS_GE, AluInp.PREV_DELAY_1, AluInp.PREV_DELAY_0)
             .enable_delay_from_src(DelayInp.PREV_ALU_OUT, 2))
    for i in range(2, 8):
        dp[i] = UopDpConfig().pass_through_delay(2)
        dp[i].alu_out_enable = 1
    u2p.enable_output(OutSel.DELAY_2, OutPath.WR0_LO)
    u2p.enable_output(OutSel.ALU_OUT, OutPath.WR0_HI)

    # 4x: e0..e3 = SRC_0, SRC_0_HI, SRC_1, SRC_1_HI
    u4 = base(UopConfig())
    u4.enable_input(InpSel.SRC_0, 0)        # e0 -> ALU lane
    u4.enable_input(InpSel.CONST_0, 1)      # th -> d0
    u4.enable_input(InpSel.SRC_0_HI, 2)     # e1 -> d1
    u4.enable_input(InpSel.SRC_1, 3)        # e2 -> d2
    u4.enable_input(InpSel.SRC_1_HI, 4)     # e3 -> d3
    dp = u4.datapath_config
    dp[0] = (UopDpConfig()
             .enable_alu(AluOp.IS_GE, AluInp.PREV_ALU_OUT, AluInp.PREV_DELAY_0)
             .pass_through_delay(0, 1, 2, 3))
    dp[1] = (UopDpConfig()
             .enable_alu(AluOp.IS_GE, AluInp.PREV_DELAY_1, AluInp.PREV_DELAY_0)
             .enable_delay_from_src(DelayInp.PREV_ALU_OUT, 4)
             .pass_through_delay(0, 2, 3))
    dp[2] = (UopDpConfig()
             .enable_alu(AluOp.IS_GE, AluInp.PREV_DELAY_2, AluInp.PREV_DELAY_0)
             .enable_delay_from_src(DelayInp.PREV_ALU_OUT, 5)
             .pass_through_delay(0, 3, 4))
    dp[3] = (UopDpConfig()
             .enable_alu(AluOp.IS_GE, AluInp.PREV_DELAY_3, AluInp.PREV_DELAY_0)
             .enable_delay_from_src(DelayInp.PREV_ALU_OUT, 0)
             .pass_through_delay(4, 5))
    for i in range(4, 8):
        dp[i] = UopDpConfig().pass_through_delay(0, 4, 5)
        dp[i].alu_out_enable = 1
    u4.enable_output(OutSel.DELAY_4, OutPath.WR0_LO)   # e0 result
    u4.enable_output(OutSel.DELAY_5, OutPath.WR0_HI)   # e1 result
    u4.enable_output(OutSel.DELAY_0, OutPath.WR1_LO)   # e2 result
    u4.enable_output(OutSel.ALU_OUT, OutPath.WR1_HI)   # e3 result
    return u1, u2, u2p, u4


@dataclass(frozen=True)
class _DveOpHand(DveOp):
    """DveOp whose compiled DveOpSpec is prebuilt (with perf-mode uops)."""
    handspec: DveOpSpec | None = None

    def compile(self, ver):
        assert ver == "v3", "hand uop built for v3/TRN2 only"
        return self.handspec


_LIF_SPEC = Spec(
    body=Src0 + select(Src1 < C1, Src1, Zero) * C0,
    reference=lambda in0, in1, s0, s1, imm2: (
        in0.astype(np.float32)
        + np.where(in1.astype(np.float32) < s1,
                   in1.astype(np.float32), np.float32(0)) * np.float32(s0)
    ),
)


_SPIKE_SPEC = Spec(
    body=Src0 >= C0,
    reference=lambda in0, in1, s0, s1, imm2: (
        in0.astype(np.float32) >= s0).astype(np.float32),
)


def _register_hand(name) -> DveOp:
    if name in dve_ops._SUB_OPCODE_FOR_NAME:
        for op in dve_ops.OPS:
            if op.name == name:
                return op
    row = dve_ops._CUSTOM_DVE_ROW_BASE + len(dve_ops.OPS)
    assert row < 0x20, "custom DVE opcode rows exhausted"
    if name == "LIF2X":
        spec, rd1 = _LIF_SPEC, True
        hs = DveOpSpec(name=name, opcode=row, uops=lower(spec, ver="v3"),
                       uops_2x=[_build_2x_uop()], perf_max=1, rd1_en=True)
    else:
        # perf_max=1: the 2-port single-src modes (2x_2P/4x) hang on HW --
        # the CUSTOM_DVE_ANT instruction carries no src1 AP for the engine
        # to drive port 1 with. 2x_1P is validated.
        spec, rd1 = _SPIKE_SPEC, False
        u1, u2, u2p, u4 = _spike_uops()
        hs = DveOpSpec(name=name, opcode=row, uops=[u1], uops_2x=[u2],
                       perf_max=1, rd1_en=False)
    op = _DveOpHand(name, spec, subdim=False, uops_sha={}, handspec=hs)
    dve_ops.OPS.append(op)
    dve_ops._SUB_OPCODE_FOR_NAME[name] = row
    dve_ops.CUSTOM_DVE_SPECS[name] = spec
    return op


def _emit_hand(vec, op, *, out, in0, in1=None, s0=0.0, s1=0.0):
    """nc.vector._custom_dve with the op's perf_max on the instruction."""
    bass = vec.bass
    if op.name not in bass.m.ant_custom_dve_ops:
        bass.m.ant_custom_dve_ops = sorted(
            {*bass.m.ant_custom_dve_ops, op.name})
    from concourse.dve_ops import get_dve_sub_opcode
    shape = bass_isa.CustomDveShape.TTSS
    isa_opcode = bass.isa.Opcode[
        f"NEURON_ISA_TPB_OPCODE_CUSTOM_DVE_ANT_{shape.slot()}"].value

    def imm(v):
        return mybir.ImmediateValue(dtype=mybir.dt.float32, value=float(v))

    ins = [vec.lower_ap(in0, for_isa=True, opt=True)]
    if in1 is not None:
        ins.append(vec.lower_ap(in1, for_isa=True, opt=True))
    ins += [imm(s0), imm(s1)]
    outs = [vec.lower_ap(out, for_isa=True, opt=True)]
    return vec.add_instruction(
        bass_isa.InstCustomDveAnt(
            name=bass.get_next_instruction_name(),
            op_name=op.name,
            rd1_en=in1 is not None,
            subdim=0,
            imm2=0.0,
            shape=shape,
            row=get_dve_sub_opcode(op.name),
            isa_opcode=isa_opcode,
            ins=ins,
            outs=outs,
            perf_max=op.handspec.perf_max,
        )
    )


# --------------------------------------------------------------------------
# Per-core program
# --------------------------------------------------------------------------

_NC_CACHE = {}


def _build_nc(repeat: int = 1, mode: str = "full", conv: str = "act",
              sign: str = "bdve"):
    """mode: full | nosign (skip signs+pack) | noscan (loads+converts only).
    conv: split | dve | act  -- where the fp32->int16 convert runs.
    sign: act (ScalarE Sign, +-1) | dve (SPIKE2X is_ge, {0,1})
          | mix (dir1 on DVE, dir2 on ACT)."""
    key = (repeat, mode, conv, sign)
    if key in _NC_CACHE:
        return _NC_CACHE[key]
    op = _register_hand("LIF2X")
    spk = _register_hand("SPIKE2X")
    nc = bacc.Bacc("TRN2", target_bir_lowering=False, debug=False,
                   num_devices=NCORES)
    x_d = nc.dram_tensor("x", [T * NCHUNK * P, CHUNK], F32,
                         kind="ExternalInput").ap()
    o_d = nc.dram_tensor("o", [NGROUP * P, FREE], U8,
                         kind="ExternalOutput").ap()

    ub = 2 if sign == "pair" else 3
    gb = 4 if sign == "pair" else 5
    xb = 3 if sign == "pair" else 4
    hb = 17 if sign == "pair" else 18
    with tile.TileContext(nc) as tc:
        with tc.tile_pool(name="xf", bufs=xb) as xfp, \
             tc.tile_pool(name="xh", bufs=hb) as xhp, \
             tc.tile_pool(name="u1", bufs=ub) as u1p, \
             tc.tile_pool(name="u2", bufs=ub) as u2p, \
             tc.tile_pool(name="g1", bufs=gb) as g1p, \
             tc.tile_pool(name="g2", bufs=gb) as g2p, \
             tc.tile_pool(name="ou", bufs=3) as oup, \
             tc.tile_pool(name="ps", bufs=2, space="PSUM") as psp, \
             tc.tile_pool(name="cn", bufs=1) as cnp:
            # constants: scaled identities 4^j (and half-weights for the
            # +-1-sign path), sign biases
            ident = cnp.tile([P, P], BF16, tag="id", name="id")
            make_identity(nc, ident[:])
            idw, idh = [ident], []
            for j in (1, 2, 3):
                t_ = cnp.tile([P, P], BF16, tag=f"id{j}", name=f"id{j}")
                nc.vector.tensor_scalar(out=t_[:], in0=ident[:],
                                        scalar1=float(4 ** j), scalar2=None,
                                        op0=mybir.AluOpType.mult)
                idw.append(t_)
            for j in range(4):
                t_ = cnp.tile([P, P], BF16, tag=f"ih{j}", name=f"ih{j}")
                nc.vector.tensor_scalar(out=t_[:], in0=ident[:],
                                        scalar1=float(4 ** j) / 2,
                                        scalar2=None,
                                        op0=mybir.AluOpType.mult)
                idh.append(t_)
            # -0.5 shift: u_hat is integer-valued, so a half-step offset makes
            # Sign() tie-free ([u >= 4096] == [u >= 4095.5]) without changing
            # the decision; exact ties at integer thresholds would emit
            # sign=0 and corrupt the packed base-4 digits.
            b1 = cnp.tile([P, 1], F32, tag="b1", name="b1")
            nc.vector.memset(b1[:], -(TH1H - 0.5))
            b2 = cnp.tile([P, 1], F32, tag="b2", name="b2")
            nc.vector.memset(b2[:], -TH2H)

            for rep in range(repeat):
                for k in range(NCHUNK):
                    c0 = k * CHUNK
                    # load + convert, in first-use order
                    xh = {}
                    for i, t in enumerate(
                            v for s in range(T // 2) for v in (s, T - 1 - s)):
                        xf = xfp.tile([P, CHUNK], F32, tag="xf",
                                      name=f"xf{rep}_{k}_{t}")
                        r0 = (t * NCHUNK + k) * P
                        nc.sync.dma_start(
                            out=xf[:], in_=x_d[r0:r0 + P, :])
                        xh[t] = xhp.tile([P, CHUNK], I16, tag="xh",
                                         name=f"xh{rep}_{k}_{t}")
                        on_dve = (conv == "dve" or
                                  (conv == "split" and i % 2 == 0))
                        if on_dve:
                            nc.vector.tensor_scalar(
                                out=xh[t][:], in0=xf[:], scalar1=SC,
                                scalar2=None, op0=mybir.AluOpType.mult)
                        else:
                            nc.scalar.activation(
                                out=xh[t][:], in_=xf[:], func=AF.Copy,
                                bias=0.0, scale=SC)
                    if mode == "noscan":
                        ob = oup.tile([P, CHUNK], U8, tag="ou", name="ou")
                        nc.vector.tensor_scalar(
                            out=ob[:, :64], in0=xh[0][:, :64], scalar1=1.0,
                            scalar2=None, op0=mybir.AluOpType.mult)
                        nc.sync.dma_start(out=o_d[:P, c0:c0 + 64],
                                          in_=ob[:, :64])
                        continue
                    if sign == "pair":
                        # double-width state tiles; one FD-4096 is_ge covers
                        # two timesteps' spikes per direction.
                        sgA, sgB = {}, {}
                        packed = set()

                        def spike(dst, src, th):
                            nc.vector.tensor_scalar(
                                out=dst, in0=src, scalar1=th, scalar2=None,
                                op0=mybir.AluOpType.is_ge)

                        def try_pack():
                            for g in range(NGROUP):
                                if g in packed:
                                    continue
                                ts_ = range(g * NGROUP, (g + 1) * NGROUP)
                                if not all(q in sgA and q in sgB
                                           for q in ts_):
                                    continue
                                packed.add(g)
                                ps = psp.tile([P, CHUNK], F32, tag="ps",
                                              name="ps")
                                for blk in range(CHUNK // 512):
                                    for j in range(NGROUP):
                                        tj = g * NGROUP + j
                                        for dmap, st_, sp_ in (
                                                (sgA, j == 0, False),
                                                (sgB, False,
                                                 j == NGROUP - 1)):
                                            tl, off = dmap[tj]
                                            b0 = off + blk * 512
                                            nc.tensor.matmul(
                                                ps[:, blk * 512:
                                                   (blk + 1) * 512],
                                                idw[j][:],
                                                tl[:, b0:b0 + 512],
                                                start=st_, stop=sp_)
                                ob = oup.tile([P, CHUNK], U8, tag="ou",
                                              name="ou")
                                nc.scalar.activation(
                                    out=ob[:], in_=ps[:], func=AF.Copy,
                                    bias=0.0, scale=1.0)
                                nc.sync.dma_start(
                                    out=o_d[g * P:(g + 1) * P,
                                            c0:c0 + CHUNK],
                                    in_=ob[:])

                        u1ap, u2ap = xh[0][:], xh[T - 1][:]
                        sa = g1p.tile([P, 2 * CHUNK], BF16, tag="g1",
                                      name="g1s0")
                        spike(sa[:, :CHUNK], u1ap, TH1H - 0.5)
                        sgA[0] = (sa, 0)
                        sb = g2p.tile([P, 2 * CHUNK], BF16, tag="g2",
                                      name="g2s0")
                        spike(sb[:, :CHUNK], u2ap, TH2H)
                        sgB[0] = (sb, 0)
                        p1 = p2 = None
                        for t in range(1, T):
                            half = (t - 1) % 2
                            if t == T - 1 or half == 0:
                                p1 = u1p.tile([P, 2 * CHUNK], I16, tag="u1",
                                              name="u1")
                                p2 = u2p.tile([P, 2 * CHUNK], I16, tag="u2",
                                              name="u2")
                                d1, d2 = p1[:, :CHUNK], p2[:, :CHUNK]
                            else:
                                d1, d2 = p1[:, CHUNK:], p2[:, CHUNK:]
                            _emit_hand(nc.vector, op, out=d1, in0=xh[t][:],
                                       in1=u1ap, s0=CC, s1=TH1H)
                            _emit_hand(nc.vector, op, out=d2,
                                       in0=xh[T - 1 - t][:], in1=u2ap,
                                       s0=CC, s1=TH2H)
                            u1ap, u2ap = d1, d2
                            if t == T - 1:
                                sa = g1p.tile([P, 2 * CHUNK], BF16,
                                              tag="g1", name="g1f")
                                spike(sa[:, :CHUNK], d1, TH1H - 0.5)
                                sgA[t] = (sa, 0)
                                sb = g2p.tile([P, 2 * CHUNK], BF16,
                                              tag="g2", name="g2f")
                                spike(sb[:, :CHUNK], d2, TH2H)
                                sgB[t] = (sb, 0)
                            elif half == 1:
                                sa = g1p.tile([P, 2 * CHUNK], BF16,
                                              tag="g1", name="g1")
                                spike(sa[:, :], p1[:, :], TH1H - 0.5)
                                sgA[t - 1] = (sa, 0)
                                sgA[t] = (sa, CHUNK)
                                sb = g2p.tile([P, 2 * CHUNK], BF16,
                                              tag="g2", name="g2")
                                spike(sb[:, :], p2[:, :], TH2H)
                                sgB[t - 1] = (sb, 0)
                                sgB[t] = (sb, CHUNK)
                            try_pack()
                        continue
                    # scans + signs; u at t=0 is the converted x tile itself
                    sg1, sg2 = {}, {}
                    u1, u2 = xh[0], xh[T - 1]
                    for t in range(T):
                        if t > 0:
                            u1n = u1p.tile([P, CHUNK], I16, tag="u1",
                                           name="u1")
                            u2n = u2p.tile([P, CHUNK], I16, tag="u2",
                                           name="u2")
                            _emit_hand(nc.vector, op, out=u1n[:],
                                       in0=xh[t][:], in1=u1[:],
                                       s0=CC, s1=TH1H)
                            _emit_hand(nc.vector, op, out=u2n[:],
                                       in0=xh[T - 1 - t][:], in1=u2[:],
                                       s0=CC, s1=TH2H)
                            u1, u2 = u1n, u2n
                        if mode == "nosign":
                            if t == T - 1:
                                ob = oup.tile([P, CHUNK], U8, tag="ou",
                                              name="ou")
                                nc.vector.tensor_tensor(
                                    out=ob[:, :64], in0=u1[:, :64],
                                    in1=u2[:, :64],
                                    op=mybir.AluOpType.add)
                                nc.sync.dma_start(out=o_d[:P, c0:c0 + 64],
                                                  in_=ob[:, :64])
                            continue
                        d1_dve = sign in ("dve", "mix", "bdve", "bmix",
                                          "gmix", "gmix2", "tune")
                        # tune: dir2 spikes of group 1 go to ACT Sign (+-1
                        # digits handled by per-group weights + copy bias).
                        d2_dve = (sign in ("dve", "bdve", "gmix", "gmix2") or
                                  (sign == "tune" and t // NGROUP != 1))
                        builtin = sign in ("bdve", "bmix", "gmix", "gmix2",
                                           "tune")

                        def emit_spike(dst, src, th, eng=None):
                            if builtin:
                                (eng or nc.vector).tensor_scalar(
                                    out=dst, in0=src, scalar1=th,
                                    scalar2=None,
                                    op0=mybir.AluOpType.is_ge)
                            else:
                                _emit_hand(nc.vector, spk, out=dst,
                                           in0=src, s0=th)
                        sg1[t] = g1p.tile([P, CHUNK], BF16, tag="g1",
                                          name="g1")
                        if d1_dve:
                            emit_spike(sg1[t][:], u1[:], TH1H - 0.5)
                        else:
                            nc.scalar.activation(out=sg1[t][:], in_=u1[:],
                                                 func=AF.Sign, bias=b1[:],
                                                 scale=1.0)
                        sg2[t] = g2p.tile([P, CHUNK], BF16, tag="g2",
                                          name="g2")
                        if sign == "gmix" or (sign == "gmix2" and t % 2 == 0):
                            emit_spike(sg2[t][:], u2[:], TH2H, nc.gpsimd)
                        elif d2_dve:
                            emit_spike(sg2[t][:], u2[:], TH2H)
                        else:
                            nc.scalar.activation(out=sg2[t][:], in_=u2[:],
                                                 func=AF.Sign, bias=b2[:],
                                                 scale=1.0)
                        if t % NGROUP == NGROUP - 1:
                            g = t // NGROUP
                            g2_dve = (sign in ("dve", "bdve", "gmix",
                                               "gmix2") or
                                      (sign == "tune" and g != 1))
                            w1 = idw if d1_dve else idh
                            w2 = idw if g2_dve else idh
                            obias = (0.0 if d1_dve else 42.5) + \
                                (0.0 if g2_dve else 42.5)
                            ps = psp.tile([P, CHUNK], F32, tag="ps",
                                          name="ps")
                            for blk in range(CHUNK // 512):
                                sl = slice(blk * 512, (blk + 1) * 512)
                                for j in range(NGROUP):
                                    tj = g * NGROUP + j
                                    nc.tensor.matmul(
                                        ps[:, sl], w1[j][:], sg1[tj][:, sl],
                                        start=(j == 0), stop=False)
                                    nc.tensor.matmul(
                                        ps[:, sl], w2[j][:], sg2[tj][:, sl],
                                        start=False, stop=(j == NGROUP - 1))
                            ob = oup.tile([P, CHUNK], U8, tag="ou", name="ou")
                            nc.scalar.activation(out=ob[:], in_=ps[:],
                                                 func=AF.Copy, bias=obias,
                                                 scale=1.0)
                            nc.sync.dma_start(
                                out=o_d[g * P:(g + 1) * P, c0:c0 + CHUNK],
                                in_=ob[:])

    nc.compile()
    _NC_CACHE[key] = nc
    return nc


def make_shard(inputs: np.ndarray, c: int) -> np.ndarray:
    """Per-core DRAM image: [T*NCHUNK*P, CHUNK] fp32 with each (t, chunk)
    tile a fully contiguous 1 MB block (contiguous DMA bursts)."""
    return np.ascontiguousarray(
        inputs[:, c * BS:(c + 1) * BS, :]
        .reshape(T, P, NCHUNK, CHUNK).transpose(0, 2, 1, 3)
        .reshape(T * NCHUNK * P, CHUNK))


def _run(inputs: np.ndarray, repeat: int = 1, **kwargs):
    nc = _build_nc(repeat)
    in_maps = []
    for c in range(NCORES):
        in_maps.append({"x": make_shard(inputs, c)})
    return bass_utils.run_bass_kernel_spmd(
        nc, in_maps, core_ids=list(range(NCORES)), **kwargs)


# digit-plane decode LUTs: byte -> 0.5 * base-4 digit j
_LUT = [((np.arange(256, dtype=np.uint8) >> (2 * j)) & 3).astype(np.float32)
        * np.float32(0.5) for j in range(NGROUP)]


def kernel(inputs: np.ndarray, **kwargs) -> np.ndarray:
    inputs = np.asarray(inputs)
    assert inputs.shape == (T, B, N) and inputs.dtype == np.float32
    res = None
    err = None
    for _attempt in range(3):  # retry transient device faults
        try:
            res = _run(inputs, **kwargs)
            break
        except Exception as e:  # noqa: BLE001
            err = e
    if res is None:
        raise err
    out = np.empty((T, B, N), np.float32)
    for c in range(NCORES):
        o = res.results[c]["o"]  # [NGROUP*P, FREE] uint8
        for g in range(NGROUP):
            plane = o[g * P:(g + 1) * P, :]
            for j in range(NGROUP):
                t = g * NGROUP + j
                out[t, c * BS:(c + 1) * BS, :] = (
                    _LUT[j][plane].reshape(BS, N))
    return out
